# revision 1
# baseline (speedup 1.0000x reference)
"""Trainium2 Bass kernel for nn_AttnConv2d (attention-conv + dynamic conv + BN).

Math (per sample b):
  a1 = conv3x3(x, w1); a2 = conv3x3(x, w2); a3 = conv3x3(x, w3)     (SAME pad)
  attn[h,w,i,o] = sum_{p,q} a1[i,3p+h,3q+w] * a2[o,3p+h,3q+w]
  kern[o,:,:,:] = softmax(attn[.,.,.,o] / sqrt(Ci*9))
  av = conv3x3(a3, kern[b])                                         (per-sample kernel)
  y  = feature_map_stack(av)   (pure spatial/channel permutation)
  out = cm * x + NORM_SCALE * (y - mean_y) * rsqrt(var_y + eps)     (batch stats)

Sharding: data-parallel over batch, 1 sample per core, 8 cores.  The only
cross-core exchange is an AllReduce of the per-channel BN partial sums.

Implementation notes:
  - x arrives host-padded ([128, H+2, W+2]) so convs are 9 shifted
    accumulating matmuls into PSUM with no on-device edge handling.  The
    static convs run the data as float32r (full-rate PE); the dynamic conv
    runs bf16.
  - attention contraction (over spatial positions) needs positions on the
    partition axis: conv outputs are written subgrid-gathered to SBUF (bf16),
    PE-transposed in 128-position chunks, then one matmul per chunk
    accumulates into a persistent PSUM tile ([o, 9*128] layout; softmax is
    permutation invariant so the flatten order doesn't matter).
  - a2's output-channel order is permuted host-side (p = 32*(c%4) + c//4) so
    the feature_map_stack scatter-DMA reads contiguous partition groups.
  - feature_map_stack is folded into the DMA that writes av to a DRAM scratch
    buffer in output layout; BN application is then a per-partition scalar op.
  - BN group-of-4 partition sums are a tiny 0/1 matmul; the diagonal
    (channel -> its own parity column) selection is a mask multiply.
"""

import os
import sys

for _p in ("/opt/trn_rl_repo", "/root/.axon_site/_ro/trn_rl_repo"):
    if os.path.isdir(_p) and _p not in sys.path:
        sys.path.insert(0, _p)
        break

import numpy as np

import concourse.bass as bass
import concourse.bacc as bacc
import concourse.tile as tile
from concourse import mybir

F32 = mybir.dt.float32
F32R = mybir.dt.float32r
BF16 = mybir.dt.bfloat16

ATTN_K = 3
NH = 2
EPS = 1e-5
NORM_SCALE = 0.1816
CI = 128
CO = 128


def _rap(base, dims, off=0):
    """Raw AP on the same tensor as `base` (keeps base's partition dim)."""
    return bass.AP(tensor=base.tensor, offset=base.offset + off,
                   ap=[base.ap[0]] + [list(d) for d in dims])


def build_nc(H, W, R, n_cores, cm, level=5):
    """Build the per-core Bass kernel. R = strip rows (div by 6, even).

    level: 1=pass A only, 2=+softmax, 3=+pass B, 4=+allreduce/BN, 5=full.
    """
    assert H % R == 0 and R % 6 == 0 and W % 6 == 0
    NS = H // R                      # strips
    Wq = W // 3                      # attn subgrid cols
    P = (R // 3) * Wq                # attn positions per offset per strip
    S = H // 2                       # quadrant size of feature_map_stack
    NT = R // 2                      # psum tiles (2 rows) per strip
    PQ = (R // 2) * (W // 2)         # parity-split positions per strip
    N_TOT = float(n_cores * H * W)   # BN count per channel
    SCL = 1.0 / float(np.sqrt(CI * 9))

    nc = bacc.Bacc("TRN2", target_bir_lowering=False, debug=False,
                   num_devices=n_cores)

    x_in = nc.dram_tensor("x", [128, H + 2, W + 2], F32,
                          kind="ExternalInput").ap()   # host-padded (+1 ring)
    w1_in = nc.dram_tensor("w1t", [128, 9, 128], F32, kind="ExternalInput").ap()
    w2_in = nc.dram_tensor("w2t", [128, 9, 128], F32, kind="ExternalInput").ap()
    w3_in = nc.dram_tensor("w3t", [128, 9, 128], F32, kind="ExternalInput").ap()
    id_in = nc.dram_tensor("ident", [128, 128], BF16, kind="ExternalInput").ap()
    gp_in = nc.dram_tensor("gsum", [128, 128], F32, kind="ExternalInput").ap()
    mk_in = nc.dram_tensor("mask4", [128, 4], F32, kind="ExternalInput").ap()
    out_d = nc.dram_tensor("out", [128, H, W], F32, kind="ExternalOutput").ap()
    avp_d = nc.dram_tensor("avp", [128, H, W], BF16).ap()   # scratch, out layout

    with tile.TileContext(nc) as tc:
        consts = tc.alloc_tile_pool(name="consts", bufs=1)
        w1t = consts.tile([128, 9, 128], F32R, tag="w1t")
        w2t = consts.tile([128, 9, 128], F32R, tag="w2t")
        w3t = consts.tile([128, 9, 128], F32R, tag="w3t")
        ident = consts.tile([128, 128], BF16, tag="ident")
        gsum = consts.tile([128, 128], F32, tag="gsum")
        mask4 = consts.tile([128, 4], F32, tag="mask4")
        nc.sync.dma_start(out=w1t[:], in_=w1_in.bitcast(F32R)[:])
        nc.sync.dma_start(out=w2t[:], in_=w2_in.bitcast(F32R)[:])
        nc.sync.dma_start(out=w3t[:], in_=w3_in.bitcast(F32R)[:])
        nc.sync.dma_start(out=ident[:], in_=id_in[:])
        nc.sync.dma_start(out=gsum[:], in_=gp_in[:])
        nc.sync.dma_start(out=mask4[:], in_=mk_in[:])

        small = tc.alloc_tile_pool(name="small", bufs=1)
        stats_cols = small.tile([128, NS, 4, 2], F32, tag="stats_cols")
        sloc = small.tile([128, 8], F32, tag="sloc")
        sglob = small.tile([128, 8], F32, tag="sglob")
        scalars = small.tile([128, 16], F32, tag="scalars")
        msb = small.tile([128, 8], F32, tag="msb")
        sel = small.tile([128, 4], F32, tag="sel")

        kern_pool = tc.alloc_tile_pool(name="kern", bufs=1)
        kernT = [kern_pool.tile([128, 128], BF16, tag=f"kT{k}", name=f"kT{k}")
                 for k in range(9)]

        a3_pool = tc.alloc_tile_pool(name="a3p", bufs=1)
        a3p = a3_pool.tile([128, H + 2, W + 2], BF16, tag="a3p")
        # zero the pad border of a3p once
        nc.vector.memset(_rap(a3p[:], [[1, W + 2]]), 0.0)                       # row 0
        nc.vector.memset(_rap(a3p[:], [[1, W + 2]], (H + 1) * (W + 2)), 0.0)    # row H+1
        nc.vector.memset(_rap(a3p[:], [[W + 2, H + 2]]), 0.0)                   # col 0
        nc.vector.memset(_rap(a3p[:], [[W + 2, H + 2]], W + 1), 0.0)            # col W+1

        attn_psp = tc.alloc_tile_pool(name="attn_ps", bufs=1, space="PSUM")
        attn_ps = attn_psp.tile([128, 9 * 128], F32, tag="attn")

        # ---------------- pass A: static convs + attention accumulation ------
        pa_x = tc.alloc_tile_pool(name="pa_x", bufs=2)
        pa_g = tc.alloc_tile_pool(name="pa_g", bufs=2)
        pa_t = tc.alloc_tile_pool(name="pa_t", bufs=4)
        pa_cps = tc.alloc_tile_pool(name="pa_cps", bufs=3, space="PSUM")
        pa_tps = tc.alloc_tile_pool(name="pa_tps", bufs=2, space="PSUM")

        for s in range(NS):
            y0 = s * R
            xs = pa_x.tile([128, R + 2, W + 2], F32R, tag="xs")
            nc.sync.dma_start(out=xs[:],
                              in_=x_in.bitcast(F32R)[:, y0:y0 + R + 2, :])

            a1g = pa_g.tile([128, 9, P], BF16, tag="a1g")
            a2g = pa_g.tile([128, 9, P], BF16, tag="a2g")
            for ci, (wt, gdst) in enumerate(((w1t, a1g), (w2t, a2g), (w3t, None))):
                for t in range(NT):
                    cps = pa_cps.tile([128, 2 * W], F32, tag="cps")
                    for k in range(9):
                        dy, dx = divmod(k, 3)
                        rhs = xs[:, 2 * t + dy:2 * t + dy + 2, dx:dx + W]
                        nc.tensor.matmul(cps[:, :], wt[:, k, :], rhs,
                                         start=(k == 0), stop=(k == 8))
                    if gdst is not None:
                        # scatter rows (2t, 2t+1) into subgrid-major layout
                        ya, yb = 2 * t, 2 * t + 1
                        ha, ra = ya % 3, ya // 3
                        hb, rb = yb % 3, yb // 3
                        offa = (3 * ha) * P + ra * Wq
                        sd = (3 * hb) * P + rb * Wq - offa
                        nc.scalar.copy(
                            out=_rap(gdst[:], [[sd, 2], [P, 3], [1, Wq]], offa),
                            in_=_rap(cps[:], [[W, 2], [1, 3], [3, Wq]]))
                    else:
                        nc.scalar.copy(
                            out=a3p[:, 1 + y0 + 2 * t:1 + y0 + 2 * t + 2, 1:1 + W],
                            in_=_rap(cps[:], [[W, 2], [1, W]]))
            # attention: transpose chunks and accumulate
            for hw in range(9):
                for c0 in range(0, P, 128):
                    ch = min(128, P - c0)
                    t1 = pa_tps.tile([128, 128], BF16, tag="tps")
                    nc.tensor.transpose(t1[0:ch, :], a1g[:, hw, c0:c0 + ch], ident[:])
                    a1T = pa_t.tile([128, 128], BF16, tag="aT")
                    nc.vector.tensor_copy(a1T[0:ch, :], t1[0:ch, :])
                    t2 = pa_tps.tile([128, 128], BF16, tag="tps")
                    nc.tensor.transpose(t2[0:ch, :], a2g[:, hw, c0:c0 + ch], ident[:])
                    a2T = pa_t.tile([128, 128], BF16, tag="aT")
                    nc.vector.tensor_copy(a2T[0:ch, :], t2[0:ch, :])
                    nc.tensor.matmul(
                        attn_ps[:, hw * 128:(hw + 1) * 128],
                        a2T[0:ch, :], a1T[0:ch, :],
                        start=(s == 0 and c0 == 0 and hw in (0, 4, 8)),
                        stop=(s == NS - 1 and c0 + 128 >= P and hw in (3, 7, 8)),
                        skip_group_check=True)

        pa_tps.release(); pa_cps.release()
        pa_t.release(); pa_g.release(); pa_x.release()

        # ---------------- softmax + kern transposes -------------------------
        if level >= 2:
            sm_pool = tc.alloc_tile_pool(name="smx", bufs=1)
            attn_sb = sm_pool.tile([128, 9 * 128], F32, tag="attn_sb")
            nc.vector.tensor_copy(attn_sb[:], attn_ps[:])
            attn_psp.release()
            k_tps = tc.alloc_tile_pool(name="k_tps", bufs=2, space="PSUM")
            mx = scalars[:, 0:1]
            nmx = scalars[:, 1:2]
            ssum = scalars[:, 2:3]
            rsum = scalars[:, 3:4]
            nc.vector.reduce_max(mx, attn_sb[:], axis=mybir.AxisListType.X)
            nc.vector.tensor_scalar_mul(nmx, mx, -SCL)
            esb = sm_pool.tile([128, 9 * 128], F32, tag="esb")
            nc.scalar.activation(esb[:], attn_sb[:],
                                 mybir.ActivationFunctionType.Exp,
                                 bias=nmx, scale=SCL)
            nc.vector.reduce_sum(ssum, esb[:], axis=mybir.AxisListType.X)
            nc.vector.reciprocal(rsum, ssum)
            sm_bf = sm_pool.tile([128, 9 * 128], BF16, tag="sm_bf")
            nc.vector.tensor_scalar_mul(sm_bf[:], esb[:], rsum)
            for hw in range(9):
                tp = k_tps.tile([128, 128], BF16, tag="ktp")
                nc.tensor.transpose(tp[:], sm_bf[:, hw * 128:(hw + 1) * 128],
                                    ident[:])
                nc.vector.tensor_copy(kernT[hw][:], tp[:])
            k_tps.release(); sm_pool.release()
        else:
            attn_psp.release()

        # ---------------- pass B: dynamic conv + stats + permuted store -----
        if level >= 3:
            pb_av = tc.alloc_tile_pool(name="pb_av", bufs=2)
            pb_sq = tc.alloc_tile_pool(name="pb_sq", bufs=2)
            pb_cps = tc.alloc_tile_pool(name="pb_cps", bufs=3, space="PSUM")
            for s in range(NS):
                y0 = s * R
                # av parity-split: av_sp[c, 2i+j, p, q] = av[c, 2p+i, 2q+j]
                av_sp = pb_av.tile([128, 4, R // 2, W // 2], BF16, tag="av")
                for t in range(NT):
                    cps = pb_cps.tile([128, 2 * W], F32, tag="cps2")
                    for k in range(9):
                        dy, dx = divmod(k, 3)
                        rhs = a3p[:, y0 + 2 * t + dy:y0 + 2 * t + dy + 2,
                                  dx:dx + W]
                        nc.tensor.matmul(cps[:, :], kernT[k][:], rhs,
                                         start=(k == 0), stop=(k == 8))
                    nc.scalar.copy(
                        out=_rap(av_sp[:], [[2 * PQ, 2], [PQ, 2], [1, W // 2]],
                                 t * (W // 2)),
                        in_=_rap(cps[:], [[W, 2], [1, 2], [2, W // 2]]))
                sq = pb_sq.tile([128, PQ], BF16, tag="sq")
                PB = os.environ.get("PB_PARTS", "sqr")
                for pi, (i, j) in enumerate(((0, 0), (0, 1), (1, 0), (1, 1))):
                    psrc = _rap(av_sp[:], [[1, PQ]], pi * PQ)
                    if "s" in PB:
                        nc.vector.reduce_sum(stats_cols[:, s, pi, 0:1], psrc,
                                             axis=mybir.AxisListType.X)
                    if "q" in PB:
                        nc.scalar.activation(
                            out=sq[:], in_=psrc,
                            func=mybir.ActivationFunctionType.Square,
                            accum_out=stats_cols[:, s, pi, 1:2])
                    if "r" not in PB:
                        continue
                    # feature_map_stack fold: av[32*c2+c1, 2p+i, 2q+j]
                    #   -> avp[4*c1 + 2i+j, S*(c2>>1) + pg, S*(c2&1) + q]
                    for c2 in range(4):
                        qsrc = _rap(av_sp[32 * c2:32 * (c2 + 1)],
                                    [[W // 2, R // 2], [1, W // 2]], pi * PQ)
                        dst = bass.AP(
                            tensor=avp_d.tensor,
                            offset=((2 * i + j) * H * W + (c2 >> 1) * S * W
                                    + (c2 & 1) * S + (s * R // 2) * W),
                            ap=[[4 * H * W, 32], [W, R // 2], [1, W // 2]])
                        nc.sync.dma_start(out=dst, in_=qsrc)
            pb_cps.release()
            pb_sq.release(); pb_av.release()
        a3_pool.release(); kern_pool.release()

        # ---------------- AllReduce of BN partial sums ----------------------
        if level >= 4:
            nc.vector.reduce_sum(
                _rap(sloc[:], [[2, 4], [1, 2]]),
                _rap(stats_cols[:], [[2, 4], [1, 2], [8, NS]]),
                axis=mybir.AxisListType.X)
            cc_pool = tc.alloc_tile_pool(name="ccd", bufs=1, space="DRAM")
            cc_in = cc_pool.tile([128, 8], F32, tag="cc_in")
            cc_out = cc_pool.tile([128, 8], F32, tag="cc_out")
            nc.gpsimd.dma_start(out=cc_in[:], in_=sloc[:])
            nc.gpsimd.collective_compute(
                "AllReduce", mybir.AluOpType.add,
                replica_groups=[list(range(n_cores))],
                ins=[cc_in.opt()], outs=[cc_out.opt()])
            nc.gpsimd.dma_start(out=sglob[:], in_=cc_out[:])

            # ------------ BN coefficients (per out-channel) -----------------
            bn_ps = tc.alloc_tile_pool(name="bn_ps", bufs=1, space="PSUM")
            gps = bn_ps.tile([128, 8], F32, tag="gps")
            nc.tensor.matmul(gps[:], gsum[:], sglob[:], start=True, stop=True)
            nc.vector.tensor_copy(msb[:], gps[:])
            bn_ps.release()
            mean = scalars[:, 4:5]
            e2 = scalars[:, 5:6]
            msq = scalars[:, 6:7]
            var = scalars[:, 7:8]
            sd = scalars[:, 8:9]
            rstd = scalars[:, 9:10]
            sc = scalars[:, 10:11]
            bb0 = scalars[:, 11:12]
            bb = scalars[:, 12:13]
            nc.vector.tensor_mul(sel[:], _rap(msb[:], [[2, 4]]), mask4[:])
            nc.vector.reduce_sum(mean, sel[:], axis=mybir.AxisListType.X)
            nc.vector.tensor_scalar_mul(mean, mean, 1.0 / N_TOT)
            nc.vector.tensor_mul(sel[:], _rap(msb[:], [[2, 4]], 1), mask4[:])
            nc.vector.reduce_sum(e2, sel[:], axis=mybir.AxisListType.X)
            nc.vector.tensor_scalar_mul(e2, e2, 1.0 / N_TOT)
            nc.vector.tensor_mul(msq, mean, mean)
            nc.vector.tensor_tensor(out=var, in0=e2, in1=msq,
                                    op=mybir.AluOpType.subtract)
            eps_ap = scalars[:, 13:14]
            nc.vector.memset(eps_ap, EPS)
            nc.scalar.activation(sd, var, mybir.ActivationFunctionType.Sqrt,
                                 bias=eps_ap)
            nc.vector.reciprocal(rstd, sd)
            nc.vector.tensor_scalar_mul(sc, rstd, NORM_SCALE)
            nc.vector.tensor_mul(bb0, mean, sc)
            nc.vector.tensor_scalar_mul(bb, bb0, -1.0)

        # ---------------- pass C: out = cm*x + sc*avp + bb ------------------
        if level >= 5:
            pc_x = tc.alloc_tile_pool(name="pc_x", bufs=3)
            pc_a = tc.alloc_tile_pool(name="pc_a", bufs=3)
            pc_t = tc.alloc_tile_pool(name="pc_t", bufs=2)
            pc_o = tc.alloc_tile_pool(name="pc_o", bufs=2)
            for s in range(NS):
                y0 = s * R
                x_s = pc_x.tile([128, R * W], F32, tag="x_s")
                nc.sync.dma_start(
                    out=x_s[:],
                    in_=x_in[:, 1 + y0:1 + y0 + R, 1:1 + W])
                av_s = pc_a.tile([128, R * W], BF16, tag="av_s")
                nc.sync.dma_start(out=av_s[:], in_=avp_d[:, y0:y0 + R, :])
                t_s = pc_t.tile([128, R * W], F32, tag="t_s")
                nc.vector.tensor_scalar(out=t_s[:], in0=av_s[:], scalar1=sc,
                                        scalar2=bb, op0=mybir.AluOpType.mult,
                                        op1=mybir.AluOpType.add)
                xc = pc_t.tile([128, R * W], F32, tag="xc")
                nc.scalar.mul(xc[:], x_s[:], float(cm))
                o_s = pc_o.tile([128, R * W], F32, tag="o_s")
                nc.vector.tensor_add(o_s[:], t_s[:], xc[:])
                nc.sync.dma_start(out=out_d[:, y0:y0 + R, :], in_=o_s[:])
            pc_o.release(); pc_t.release(); pc_a.release(); pc_x.release()

        if level >= 4:
            cc_pool.release()
        small.release()
        consts.release()

    nc.compile()
    return nc


def _prep_wt(w, permute_out=False):
    """[Co,Ci,3,3] -> lhsT layout [Ci, 9, Co] (optionally out-chan permuted)."""
    wt = np.ascontiguousarray(w.transpose(1, 2, 3, 0).reshape(128, 9, 128))
    if permute_out:
        p = np.arange(128)
        co_of_p = 4 * (p % 32) + p // 32     # partition p holds channel co_of_p
        wt = np.ascontiguousarray(wt[:, :, co_of_p])
    return wt


def make_const_inputs(w1, w2, w3):
    import ml_dtypes
    ident_bf = np.eye(128, dtype=np.float32).astype(ml_dtypes.bfloat16)
    p = np.arange(128)
    # gsum[p_src, C']: sum av partitions with p_src%32 == C'//4
    gsum = (p[:, None] % 32 == p[None, :] // 4).astype(np.float32)
    mask4 = (p[:, None] % 4 == np.arange(4)[None, :]).astype(np.float32)
    return {
        "w1t": _prep_wt(np.asarray(w1, np.float32)),
        "w2t": _prep_wt(np.asarray(w2, np.float32), permute_out=True),
        "w3t": _prep_wt(np.asarray(w3, np.float32)),
        "ident": ident_bf,
        "gsum": gsum,
        "mask4": mask4,
    }


def pad_x(x_sample):
    return np.pad(x_sample, ((0, 0), (1, 1), (1, 1)))


_CACHE = {}


def kernel(x, w1, w2, w3, conv_momentum):
    from concourse.bass_utils import run_bass_kernel_spmd

    x = np.asarray(x, np.float32)
    B, Ci, H, W = x.shape
    cm = float(np.asarray(conv_momentum))
    key = (H, W, B, cm)
    if key not in _CACHE:
        _CACHE[key] = build_nc(H, W, 24, B, cm)
    nc = _CACHE[key]
    consts = make_const_inputs(w1, w2, w3)
    in_maps = [dict(consts, x=pad_x(x[b])) for b in range(B)]
    res = run_bass_kernel_spmd(nc, in_maps, list(range(B)))
    out = np.stack(
        [np.asarray(res.results[b]["out"]).reshape(128, H, W) for b in range(B)],
        axis=0)
    return out.astype(np.float32)



# revision 9
# speedup vs baseline: 1.1659x; 1.1659x over previous
"""Trainium2 Bass kernel for nn_AttnConv2d (attention-conv + dynamic conv + BN).

Math (per sample b):
  a1 = conv3x3(x, w1); a2 = conv3x3(x, w2); a3 = conv3x3(x, w3)     (SAME pad)
  attn[h,w,i,o] = sum_{p,q} a1[i,3p+h,3q+w] * a2[o,3p+h,3q+w]
  kern[o,:,:,:] = softmax(attn[.,.,.,o] / sqrt(Ci*9))
  av = conv3x3(a3, kern[b])                                         (per-sample kernel)
  y  = feature_map_stack(av)   (pure spatial/channel permutation)
  out = cm * x + NORM_SCALE * (y - mean_y) * rsqrt(var_y + eps)     (batch stats)

Sharding: data-parallel over batch, 1 sample per core, 8 cores.  The only
cross-core exchange is an AllReduce of the per-channel BN partial sums.

Implementation notes:
  - x arrives host-padded ([128, H+2, W+2]) in bf16 (a1/a2 convs) and fp8
    (a3 conv), so convs are 9 shifted accumulating matmuls into PSUM with
    no on-device edge handling.  a3 and the dynamic conv run fp8 with
    DoubleRow perf mode (two kernel offsets per matmul, K=256); fp8 there
    costs ~1e-2 rel err (attention path must stay bf16: fp8 a1/a2 alone
    is 1.8e-2).
  - attention contraction needs positions on the partition axis: conv
    outputs are scatter-copied to subgrid-major SBUF (bf16), PE-transposed
    in 128-position chunks, then accumulated into a persistent PSUM tile.
  - a2's output-channel order is permuted host-side (partition p holds
    channel 4*(p%32) + p//32) so feature_map_stack becomes a
    per-(partition,parity) affine map on av.
  - av stays in SBUF as 4 parity planes [128, 4, 96, 96] (no DRAM round
    trip).  BN group sums use a 0/1 matmul with gsum2[p,c] = (p%32==c%32),
    landing mean/var directly in av-partition layout (4 parity cols each).
  - pass C loads x permuted into av layout straight from DRAM, applies
    out = cm*x + sc4*av + bb4 per parity plane across scalar/vector/gpsimd,
    and folds the feature_map_stack into the store DMA.  x prefetch is
    issued before the AllReduce so the collective latency is hidden.
  - strip pipeline is skewed: transposes+attention for strip s issue after
    the convs of strip s+1, so the scalar-engine scatter never stalls PE.
"""

import os
import sys

for _p in ("/opt/trn_rl_repo", "/root/.axon_site/_ro/trn_rl_repo"):
    if os.path.isdir(_p) and _p not in sys.path:
        sys.path.insert(0, _p)
        break

import numpy as np

import concourse.bass as bass
import concourse.bacc as bacc
import concourse.tile as tile
from concourse import mybir

F32 = mybir.dt.float32
BF16 = mybir.dt.bfloat16
FP8 = mybir.dt.float8e4
DR = mybir.MatmulPerfMode.DoubleRow

EPS = 1e-5
NORM_SCALE = 0.1816
CI = 128

# a3/dynamic-conv path runs fp8 (attention path stays bf16); DR_CONV
# selects DoubleRow pairing (2 offsets per matmul) vs plain fp8 matmuls.
DR_CONV = os.environ.get("DR_CONV", "1") == "1"


def _rap(base, dims, off=0):
    """Raw AP on the same tensor as `base` (keeps base's partition dim)."""
    return bass.AP(tensor=base.tensor, offset=base.offset + off,
                   ap=[base.ap[0]] + [list(d) for d in dims])


def _conv_tile(nc, cps, wt, xs, base, W, XW, dr):
    """Accumulate the 9-offset conv into PSUM cps [128, 2*W] for the row
    pair whose top-left (unshifted) element is at linear offset `base`
    within xs (a padded [128, rows, XW] tile)."""
    if not dr:
        for k in range(9):
            dy, dx = divmod(k, 3)
            rhs = _rap(xs[:], [[XW, 2], [1, W]], base + dy * XW + dx)
            nc.tensor.matmul(cps[:, :], wt[:, k, :], rhs,
                             start=(k == 0), stop=(k == 8))
        return
    deltas = [dy * XW + dx for dy in range(3) for dx in range(3)]
    for j in range(4):
        da, db = deltas[2 * j], deltas[2 * j + 1]
        rhs = _rap(xs[:], [[db - da, 2], [XW, 2], [1, W]], base + da)
        nc.tensor.matmul(cps[:, :], wt[:, 2 * j:2 * j + 2, :], rhs,
                         start=(j == 0), stop=False, perf_mode=DR,
                         skip_group_check=True)
    rhs = _rap(xs[:], [[XW, 2], [1, W]], base + deltas[8])
    nc.tensor.matmul(cps[:, :], wt[:, 8, :], rhs,
                     start=False, stop=True, skip_group_check=True)


def build_nc(H, W, R, n_cores, cm, level=5):
    """Build the per-core Bass kernel. R = strip rows (div by 6, even)."""
    assert H % R == 0 and R % 6 == 0 and W % 6 == 0
    NS = H // R                      # strips
    Wq = W // 3                      # attn subgrid cols
    P = (R // 3) * Wq                # attn positions per offset per strip
    S = H // 2                       # quadrant size of feature_map_stack
    NT = R // 2                      # psum tiles (2 rows) per strip
    SR = R // 2                      # subgrid rows per strip (parity space)
    SQ = W // 2                      # subgrid cols (parity space)
    N_TOT = float(n_cores * H * W)   # BN count per channel
    SCL = 1.0 / float(np.sqrt(CI * 9))
    XW = W + 2                       # padded row pitch

    nc = bacc.Bacc("TRN2", target_bir_lowering=False, debug=False,
                   num_devices=n_cores)

    xb_in = nc.dram_tensor("xb", [128, H + 2, XW], BF16,
                           kind="ExternalInput").ap()
    x8_in = nc.dram_tensor("x8", [128, H + 2, XW], FP8,
                           kind="ExternalInput").ap()
    x_in = nc.dram_tensor("x", [128, H + 2, XW], F32,
                          kind="ExternalInput").ap()
    w1_in = nc.dram_tensor("w1t", [128, 9, 128], BF16,
                           kind="ExternalInput").ap()
    w2_in = nc.dram_tensor("w2t", [128, 9, 128], BF16,
                           kind="ExternalInput").ap()
    w3_in = nc.dram_tensor("w3t", [128, 9, 128], FP8,
                           kind="ExternalInput").ap()
    id_in = nc.dram_tensor("ident", [128, 128], BF16, kind="ExternalInput").ap()
    gp_in = nc.dram_tensor("gsum2", [128, 128], F32, kind="ExternalInput").ap()
    out_d = nc.dram_tensor("out", [128, H, W], F32, kind="ExternalOutput").ap()

    with tile.TileContext(nc) as tc:
        consts = tc.alloc_tile_pool(name="consts", bufs=1)
        w1t = consts.tile([128, 9, 128], BF16, tag="w1t")
        w2t = consts.tile([128, 9, 128], BF16, tag="w2t")
        w3t = consts.tile([128, 9, 128], FP8, tag="w3t")
        ident = consts.tile([128, 128], BF16, tag="ident")
        gsum2 = consts.tile([128, 128], F32, tag="gsum2")
        nc.sync.dma_start(out=w1t[:], in_=w1_in[:])
        nc.sync.dma_start(out=w2t[:], in_=w2_in[:])
        nc.sync.dma_start(out=w3t[:], in_=w3_in[:])
        nc.sync.dma_start(out=ident[:], in_=id_in[:])
        nc.sync.dma_start(out=gsum2[:], in_=gp_in[:])

        small = tc.alloc_tile_pool(name="small", bufs=1)
        stats_cols = small.tile([128, NS, 4, 2], F32, tag="stats_cols")
        sloc = small.tile([128, 8], F32, tag="sloc")
        sglob = small.tile([128, 8], F32, tag="sglob")
        scalars = small.tile([128, 16], F32, tag="scalars")
        c4 = small.tile([128, 16], F32, tag="c4")

        # av parity planes: av_full[c, 2i+j, p, q] = av[c, 2p+i, 2q+j]
        av_pool = tc.alloc_tile_pool(name="av", bufs=1)
        av_full = av_pool.tile([128, 4, S, S], BF16, tag="av_full")

        kern_pool = tc.alloc_tile_pool(name="kern", bufs=1)
        kern8 = kern_pool.tile([128, 9, 128], FP8, tag="kern8")

        a3_pool = tc.alloc_tile_pool(name="a3p", bufs=1)
        a3p = a3_pool.tile([128, H + 2, XW], FP8, tag="a3p")
        # zero the pad border of a3p once
        nc.vector.memset(_rap(a3p[:], [[1, XW]]), 0.0)                    # row 0
        nc.vector.memset(_rap(a3p[:], [[1, XW]], (H + 1) * XW), 0.0)      # row H+1
        nc.vector.memset(_rap(a3p[:], [[XW, H + 2]]), 0.0)                # col 0
        nc.vector.memset(_rap(a3p[:], [[XW, H + 2]], W + 1), 0.0)         # col W+1

        attn_psp = tc.alloc_tile_pool(name="attn_ps", bufs=1, space="PSUM")
        attn_ps = attn_psp.tile([128, 9 * 128], F32, tag="attn")

        # ---------------- pass A: static convs + attention accumulation ------
        pa_x = tc.alloc_tile_pool(name="pa_x", bufs=2)
        pa_g = tc.alloc_tile_pool(name="pa_g", bufs=2)
        pa_t = tc.alloc_tile_pool(name="pa_t", bufs=4)
        pa_cps = tc.alloc_tile_pool(name="pa_cps", bufs=3, space="PSUM")
        pa_tps = tc.alloc_tile_pool(name="pa_tps", bufs=2, space="PSUM")

        a1gs = {}
        a2gs = {}

        def conv_strip(s):
            y0 = s * R
            xsb = pa_x.tile([128, R + 2, XW], BF16, tag="xsb")
            nc.sync.dma_start(out=xsb[:], in_=xb_in[:, y0:y0 + R + 2, :])
            xs8 = pa_x.tile([128, R + 2, XW], FP8, tag="xs8")
            nc.sync.dma_start(out=xs8[:], in_=x8_in[:, y0:y0 + R + 2, :])
            a1g = pa_g.tile([128, 9, P], BF16, tag="a1g")
            a2g = pa_g.tile([128, 9, P], BF16, tag="a2g")
            a1gs[s] = a1g
            a2gs[s] = a2g
            for wt, gdst, xt, fp8 in ((w1t, a1g, xsb, False),
                                      (w2t, a2g, xsb, False),
                                      (w3t, None, xs8, DR_CONV)):
                for t in range(NT):
                    cps = pa_cps.tile([128, 2 * W], F32, tag="cps")
                    _conv_tile(nc, cps, wt, xt, 2 * t * XW, W, XW, fp8)
                    if gdst is not None:
                        # scatter rows (2t, 2t+1) into subgrid-major layout
                        ya, yb = 2 * t, 2 * t + 1
                        ha, ra = ya % 3, ya // 3
                        hb, rb = yb % 3, yb // 3
                        offa = (3 * ha) * P + ra * Wq
                        sd = (3 * hb) * P + rb * Wq - offa
                        eng_copy = (nc.scalar.copy if gdst is a1g
                                    else nc.vector.tensor_copy)
                        eng_copy(
                            _rap(gdst[:], [[sd, 2], [P, 3], [1, Wq]], offa),
                            _rap(cps[:], [[W, 2], [1, 3], [3, Wq]]))
                    else:
                        nc.scalar.copy(
                            out=a3p[:, 1 + y0 + 2 * t:1 + y0 + 2 * t + 2,
                                    1:1 + W],
                            in_=_rap(cps[:], [[W, 2], [1, W]]))

        def attn_strip(s):
            a1g, a2g = a1gs.pop(s), a2gs.pop(s)
            for hw in range(9):
                for c0 in range(0, P, 128):
                    ch = min(128, P - c0)
                    t1 = pa_tps.tile([128, 128], BF16, tag="tps")
                    nc.tensor.transpose(t1[0:ch, :], a1g[:, hw, c0:c0 + ch],
                                        ident[:])
                    a1T = pa_t.tile([128, 128], BF16, tag="aT")
                    nc.vector.tensor_copy(a1T[0:ch, :], t1[0:ch, :])
                    t2 = pa_tps.tile([128, 128], BF16, tag="tps")
                    nc.tensor.transpose(t2[0:ch, :], a2g[:, hw, c0:c0 + ch],
                                        ident[:])
                    a2T = pa_t.tile([128, 128], BF16, tag="aT")
                    nc.vector.tensor_copy(a2T[0:ch, :], t2[0:ch, :])
                    nc.tensor.matmul(
                        attn_ps[:, hw * 128:(hw + 1) * 128],
                        a2T[0:ch, :], a1T[0:ch, :],
                        start=(s == 0 and c0 == 0 and hw in (0, 4, 8)),
                        stop=(s == NS - 1 and c0 + 128 >= P and hw in (3, 7, 8)),
                        skip_group_check=True)

        # skewed pipeline: the scalar/vector scatter of strip s drains while
        # PE runs the convs of strip s+1.
        for s in range(NS):
            conv_strip(s)
            if s > 0:
                attn_strip(s - 1)
        attn_strip(NS - 1)

        pa_tps.release(); pa_cps.release()
        pa_t.release(); pa_g.release(); pa_x.release()

        # ---------------- softmax + kern transposes -------------------------
        if level >= 2:
            sm_pool = tc.alloc_tile_pool(name="smx", bufs=1)
            attn_sb = sm_pool.tile([128, 9 * 128], F32, tag="attn_sb")
            nc.vector.tensor_copy(attn_sb[:], attn_ps[:])
            attn_psp.release()
            k_tps = tc.alloc_tile_pool(name="k_tps", bufs=2, space="PSUM")
            mx = scalars[:, 0:1]
            nmx = scalars[:, 1:2]
            ssum = scalars[:, 2:3]
            rsum = scalars[:, 3:4]
            nc.vector.reduce_max(mx, attn_sb[:], axis=mybir.AxisListType.X)
            nc.vector.tensor_scalar_mul(nmx, mx, -SCL)
            esb = sm_pool.tile([128, 9 * 128], F32, tag="esb")
            nc.scalar.activation(esb[:], attn_sb[:],
                                 mybir.ActivationFunctionType.Exp,
                                 bias=nmx, scale=SCL)
            nc.vector.reduce_sum(ssum, esb[:], axis=mybir.AxisListType.X)
            nc.vector.reciprocal(rsum, ssum)
            sm_bf = sm_pool.tile([128, 9 * 128], BF16, tag="sm_bf")
            nc.vector.tensor_scalar_mul(sm_bf[:], esb[:], rsum)
            for hw in range(9):
                tp = k_tps.tile([128, 128], BF16, tag="ktp")
                nc.tensor.transpose(tp[:], sm_bf[:, hw * 128:(hw + 1) * 128],
                                    ident[:])
                nc.vector.tensor_copy(kern8[:, hw, :], tp[:])
            k_tps.release(); sm_pool.release()
        else:
            attn_psp.release()

        # ---------------- pass B: dynamic conv + stats + parity-plane store --
        if level >= 3:
            pb_sq = tc.alloc_tile_pool(name="pb_sq", bufs=2)
            pb_cps = tc.alloc_tile_pool(name="pb_cps", bufs=3, space="PSUM")

            def dyn_strip(s):
                y0 = s * R
                for t in range(NT):
                    cps = pb_cps.tile([128, 2 * W], F32, tag="cps2")
                    _conv_tile(nc, cps, kern8, a3p, (y0 + 2 * t) * XW, W, XW,
                               DR_CONV)
                    # parity-split store into av_full planes
                    nc.scalar.copy(
                        out=_rap(av_full[:], [[2 * S * S, 2], [S * S, 2],
                                              [1, SQ]], (s * SR + t) * S),
                        in_=_rap(cps[:], [[W, 2], [1, 2], [2, SQ]]))

            def stats_strip(s):
                sq = pb_sq.tile([128, SR * S], BF16, tag="sq")
                for pi in range(4):
                    psrc = _rap(av_full[:], [[1, SR * S]],
                                pi * S * S + s * SR * S)
                    nc.vector.reduce_sum(stats_cols[:, s, pi, 0:1], psrc,
                                         axis=mybir.AxisListType.X)
                    nc.scalar.activation(
                        out=sq[:], in_=psrc,
                        func=mybir.ActivationFunctionType.Square,
                        accum_out=stats_cols[:, s, pi, 1:2])

            for s in range(NS):
                dyn_strip(s)
                if s > 0:
                    stats_strip(s - 1)
            stats_strip(NS - 1)
            pb_cps.release()
            pb_sq.release()
        a3_pool.release(); kern_pool.release()

        # ---------------- AllReduce of BN partial sums ----------------------
        if level >= 4:
            # sloc cols: [par + 4*u] (sums at 0..3, sumsqs at 4..7)
            nc.vector.reduce_sum(
                _rap(sloc[:], [[1, 4], [4, 2]]),
                _rap(stats_cols[:], [[2, 4], [1, 2], [8, NS]]),
                axis=mybir.AxisListType.X)
            cc_pool = tc.alloc_tile_pool(name="ccd", bufs=1, space="DRAM")
            cc_in = cc_pool.tile([128, 8], F32, tag="cc_in")
            cc_out = cc_pool.tile([128, 8], F32, tag="cc_out")
            nc.gpsimd.dma_start(out=cc_in[:], in_=sloc[:])
            nc.gpsimd.collective_compute(
                "AllReduce", mybir.AluOpType.add,
                replica_groups=[list(range(n_cores))],
                ins=[cc_in.opt()], outs=[cc_out.opt()])
            nc.gpsimd.dma_start(out=sglob[:], in_=cc_out[:])

        # ---------------- pass C: out = cm*x + sc4*av + bb4 ------------------
        if level >= 5:
            pc_x = tc.alloc_tile_pool(name="pc_x", bufs=2)
            pc_t = tc.alloc_tile_pool(name="pc_t", bufs=2)
            pc_o = tc.alloc_tile_pool(name="pc_o", bufs=2)
            NB = S // SR                 # pass-C blocks (SR subgrid rows each)

            # prefetch permuted x blocks; independent of the collective, so
            # these DMAs hide its latency
            xps = {}
            for b2 in range(min(2, NB)):
                xps[b2] = _xp_load(nc, pc_x, x_in, b2, SR, S, W, XW)

            if level >= 4:
                # BN coefficients, already in av-partition layout
                bn_ps = tc.alloc_tile_pool(name="bn_ps", bufs=1, space="PSUM")
                gps = bn_ps.tile([128, 8], F32, tag="gps")
                nc.tensor.matmul(gps[:], gsum2[:], sglob[:], start=True,
                                 stop=True)
                mean4 = c4[:, 0:4]
                e24 = c4[:, 4:8]
                sc4 = c4[:, 8:12]
                bb4 = c4[:, 12:16]
                var4 = scalars[:, 4:8]
                eps_ap = scalars[:, 12:13]
                nc.vector.tensor_scalar_mul(mean4, gps[:, 0:4], 1.0 / N_TOT)
                nc.vector.tensor_scalar_mul(e24, gps[:, 4:8], 1.0 / N_TOT)
                bn_ps.release()
                nc.vector.tensor_tensor(out=var4, in0=mean4, in1=mean4,
                                        op=mybir.AluOpType.mult)
                nc.vector.tensor_tensor(out=var4, in0=e24, in1=var4,
                                        op=mybir.AluOpType.subtract)
                nc.vector.memset(eps_ap, EPS)
                nc.scalar.activation(var4, var4,
                                     mybir.ActivationFunctionType.Sqrt,
                                     bias=eps_ap)
                nc.vector.reciprocal(var4, var4)
                nc.vector.tensor_scalar_mul(sc4, var4, NORM_SCALE)
                nc.vector.tensor_tensor(out=bb4, in0=mean4, in1=sc4,
                                        op=mybir.AluOpType.mult)
                nc.vector.tensor_scalar_mul(bb4, bb4, -1.0)

            for b2 in range(NB):
                xp = xps.pop(b2)
                if b2 + 2 < NB:
                    xps[b2 + 2] = _xp_load(nc, pc_x, x_in, b2 + 2, SR, S, W,
                                           XW)
                ot = pc_o.tile([128, 4, SR, SQ], F32, tag="ot")
                tv = pc_t.tile([128, 4, SR, SQ], F32, tag="tv")
                for pi in range(4):
                    avb = _rap(av_full[:], [[1, SR * S]],
                               pi * S * S + b2 * SR * S)
                    tvp = _rap(tv[:], [[1, SR * SQ]], pi * SR * SQ)
                    otp = _rap(ot[:], [[1, SR * SQ]], pi * SR * SQ)
                    xpp = _rap(xp[:], [[1, SR * SQ]], pi * SR * SQ)
                    if pi < 2:
                        nc.scalar.activation(
                            tvp, avb, mybir.ActivationFunctionType.Identity,
                            bias=bb4[:, pi:pi + 1], scale=sc4[:, pi:pi + 1])
                    else:
                        nc.vector.tensor_scalar(
                            out=tvp, in0=avb, scalar1=sc4[:, pi:pi + 1],
                            scalar2=bb4[:, pi:pi + 1],
                            op0=mybir.AluOpType.mult,
                            op1=mybir.AluOpType.add)
                    nc.vector.scalar_tensor_tensor(
                        out=otp, in0=xpp, scalar=float(cm), in1=tvp,
                        op0=mybir.AluOpType.mult, op1=mybir.AluOpType.add)
                # permuted store: out[4*(c%32)+par, S*(c2>>1)+p, S*(c2&1)+q]
                for c2 in range(4):
                    src = _rap(ot[32 * c2:32 * (c2 + 1)],
                               [[SR * SQ, 4], [SQ, SR], [1, SQ]])
                    dst = bass.AP(
                        tensor=out_d.tensor,
                        offset=((c2 >> 1) * S + b2 * SR) * W + (c2 & 1) * SQ,
                        ap=[[4 * H * W, 32], [H * W, 4], [W, SR], [1, SQ]])
                    nc.sync.dma_start(out=dst, in_=src)
            pc_o.release(); pc_t.release(); pc_x.release()

        if level >= 4:
            cc_pool.release()
        av_pool.release()
        small.release()
        consts.release()

    nc.compile()
    return nc


def _xp_load(nc, pool, x_in, b2, SR, S, W, XW):
    """Load x permuted into av layout: xp[c, par, p, q] =
    x[4*(c%32)+par, 1 + S*(c//64) + b2*SR + p, 1 + S*((c//32)&1) + q]."""
    SQ = W // 2
    xp = pool.tile([128, 4, SR, SQ], F32, tag="xp")
    chan = XW * XW  # padded x is square: [128, H+2, W+2] with H == W
    for c2 in range(4):
        dst = _rap(xp[32 * c2:32 * (c2 + 1)],
                   [[SR * SQ, 4], [SQ, SR], [1, SQ]])
        src = bass.AP(
            tensor=x_in.tensor,
            offset=(1 + (c2 >> 1) * S + b2 * SR) * XW + 1 + (c2 & 1) * SQ,
            ap=[[4 * chan, 32], [chan, 4], [XW, SR], [1, SQ]])
        nc.sync.dma_start(out=dst, in_=src)
    return xp


def _prep_wt(w, dt, permute_out=False):
    """[Co,Ci,3,3] -> lhsT layout [Ci, 9, Co] (optionally out-chan permuted)."""
    wt = np.ascontiguousarray(w.transpose(1, 2, 3, 0).reshape(128, 9, 128))
    if permute_out:
        p = np.arange(128)
        co_of_p = 4 * (p % 32) + p // 32     # partition p holds channel co_of_p
        wt = np.ascontiguousarray(wt[:, :, co_of_p])
    return np.ascontiguousarray(wt.astype(dt))


def make_const_inputs(w1, w2, w3):
    import ml_dtypes
    E4 = ml_dtypes.float8_e4m3
    BF = ml_dtypes.bfloat16
    ident_bf = np.eye(128, dtype=np.float32).astype(BF)
    p = np.arange(128)
    # gsum2[p_src, c]: per-(c%32,par) group sums, summing the 4 c2 copies
    gsum2 = (p[:, None] % 32 == p[None, :] % 32).astype(np.float32)
    return {
        "w1t": _prep_wt(np.asarray(w1, np.float32), BF),
        "w2t": _prep_wt(np.asarray(w2, np.float32), BF, permute_out=True),
        "w3t": _prep_wt(np.asarray(w3, np.float32), E4),
        "ident": ident_bf,
        "gsum2": gsum2,
    }


def pad_x(x_sample):
    return np.pad(x_sample, ((0, 0), (1, 1), (1, 1)))


def make_in_maps(x, w1, w2, w3):
    import ml_dtypes
    consts = make_const_inputs(w1, w2, w3)
    in_maps = []
    for b in range(x.shape[0]):
        xp = pad_x(np.asarray(x[b], np.float32))
        m = dict(consts, x=xp,
                 xb=np.ascontiguousarray(xp.astype(ml_dtypes.bfloat16)),
                 x8=np.ascontiguousarray(xp.astype(ml_dtypes.float8_e4m3)))
        in_maps.append(m)
    return in_maps


_CACHE = {}


def kernel(x, w1, w2, w3, conv_momentum):
    from concourse.bass_utils import run_bass_kernel_spmd

    x = np.asarray(x, np.float32)
    B, Ci, H, W = x.shape
    cm = float(np.asarray(conv_momentum))
    key = (H, W, B, cm)
    if key not in _CACHE:
        _CACHE[key] = build_nc(H, W, 24, B, cm)
    nc = _CACHE[key]
    in_maps = make_in_maps(x, w1, w2, w3)
    res = run_bass_kernel_spmd(nc, in_maps, list(range(B)))
    out = np.stack(
        [np.asarray(res.results[b]["out"]).reshape(128, H, W) for b in range(B)],
        axis=0)
    return out.astype(np.float32)


# revision 11
# speedup vs baseline: 1.2888x; 1.1054x over previous
"""Trainium2 Bass kernel for nn_AttnConv2d (attention-conv + dynamic conv + BN).

Math (per sample b):
  a1 = conv3x3(x, w1); a2 = conv3x3(x, w2); a3 = conv3x3(x, w3)     (SAME pad)
  attn[h,w,i,o] = sum_{p,q} a1[i,3p+h,3q+w] * a2[o,3p+h,3q+w]
  kern[o,:,:,:] = softmax(attn[.,.,.,o] / sqrt(Ci*9))
  av = conv3x3(a3, kern[b])                                         (per-sample kernel)
  y  = feature_map_stack(av)   (pure spatial/channel permutation)
  out = cm * x + NORM_SCALE * (y - mean_y) * rsqrt(var_y + eps)     (batch stats)

Sharding: data-parallel over batch, 1 sample per core, 8 cores.  The only
cross-core exchange is an AllReduce of the per-channel BN partial sums.

Implementation notes:
  - x arrives host-padded ([128, H+2, W+2]) in bf16 (a1/a2 convs) and fp8
    (a3 conv), so convs are 9 shifted accumulating matmuls into PSUM with
    no on-device edge handling.  a3 and the dynamic conv run fp8 with
    DoubleRow perf mode (two kernel offsets per matmul, K=256); fp8 there
    costs ~1e-2 rel err (attention path must stay bf16: fp8 a1/a2 alone
    is 1.8e-2).
  - attention contraction needs positions on the partition axis: conv
    outputs are scatter-copied to subgrid-major SBUF (bf16), PE-transposed
    in 128-position chunks, then accumulated into a persistent PSUM tile.
  - a2's output-channel order is permuted host-side (partition p holds
    channel 4*(p%32) + p//32) so feature_map_stack becomes a
    per-(partition,parity) affine map on av.
  - feature_map_stack is applied strip-by-strip during pass B as
    SBUF->SBUF scatter DMAs into a y-layout tile, so the permutation's
    descriptor cost overlaps the dynamic-conv compute; pass C then runs
    entirely on clean contiguous DMAs (x strip load, out strip store).
  - BN group-of-4 partition sums are a tiny 0/1 matmul; x prefetch is
    issued before the AllReduce so the collective latency is hidden.
  - strip pipeline is skewed: transposes+attention for strip s issue after
    the convs of strip s+1, so the scalar-engine scatter never stalls PE.
"""

import os
import sys

for _p in ("/opt/trn_rl_repo", "/root/.axon_site/_ro/trn_rl_repo"):
    if os.path.isdir(_p) and _p not in sys.path:
        sys.path.insert(0, _p)
        break

import numpy as np

import concourse.bass as bass
import concourse.bacc as bacc
import concourse.tile as tile
from concourse import mybir

F32 = mybir.dt.float32
BF16 = mybir.dt.bfloat16
FP8 = mybir.dt.float8e4
DR = mybir.MatmulPerfMode.DoubleRow

EPS = 1e-5
NORM_SCALE = 0.1816
CI = 128

# a3/dynamic-conv path runs fp8 (attention path stays bf16); DR_CONV
# selects DoubleRow pairing (2 offsets per matmul) vs plain fp8 matmuls.
DR_CONV = os.environ.get("DR_CONV", "1") == "1"


def _rap(base, dims, off=0):
    """Raw AP on the same tensor as `base` (keeps base's partition dim)."""
    return bass.AP(tensor=base.tensor, offset=base.offset + off,
                   ap=[base.ap[0]] + [list(d) for d in dims])


def _conv_tile(nc, cps, wt, xs, base, W, XW, dr):
    """Accumulate the 9-offset conv into PSUM cps [128, 2*W] for the row
    pair whose top-left (unshifted) element is at linear offset `base`
    within xs (a padded [128, rows, XW] tile)."""
    if not dr:
        for k in range(9):
            dy, dx = divmod(k, 3)
            rhs = _rap(xs[:], [[XW, 2], [1, W]], base + dy * XW + dx)
            nc.tensor.matmul(cps[:, :], wt[:, k, :], rhs,
                             start=(k == 0), stop=(k == 8))
        return
    deltas = [dy * XW + dx for dy in range(3) for dx in range(3)]
    for j in range(4):
        da, db = deltas[2 * j], deltas[2 * j + 1]
        rhs = _rap(xs[:], [[db - da, 2], [XW, 2], [1, W]], base + da)
        nc.tensor.matmul(cps[:, :], wt[:, 2 * j:2 * j + 2, :], rhs,
                         start=(j == 0), stop=False, perf_mode=DR,
                         skip_group_check=True)
    rhs = _rap(xs[:], [[XW, 2], [1, W]], base + deltas[8])
    nc.tensor.matmul(cps[:, :], wt[:, 8, :], rhs,
                     start=False, stop=True, skip_group_check=True)


def build_nc(H, W, R, n_cores, cm, level=5):
    """Build the per-core Bass kernel. R = strip rows (div by 6, even)."""
    assert H % R == 0 and R % 6 == 0 and W % 6 == 0
    NS = H // R                      # strips
    Wq = W // 3                      # attn subgrid cols
    P = (R // 3) * Wq                # attn positions per offset per strip
    S = H // 2                       # quadrant size of feature_map_stack
    NT = R // 2                      # psum tiles (2 rows) per strip
    SR = R // 2                      # subgrid rows per strip (parity space)
    SQ = W // 2                      # subgrid cols (parity space)
    N_TOT = float(n_cores * H * W)   # BN count per channel
    SCL = 1.0 / float(np.sqrt(CI * 9))
    XW = W + 2                       # padded row pitch

    nc = bacc.Bacc("TRN2", target_bir_lowering=False, debug=False,
                   num_devices=n_cores)

    xb_in = nc.dram_tensor("xb", [128, H + 2, XW], BF16,
                           kind="ExternalInput").ap()
    x8_in = nc.dram_tensor("x8", [128, H + 2, XW], FP8,
                           kind="ExternalInput").ap()
    x_in = nc.dram_tensor("x", [128, H + 2, XW], F32,
                          kind="ExternalInput").ap()
    w1_in = nc.dram_tensor("w1t", [128, 9, 128], BF16,
                           kind="ExternalInput").ap()
    w2_in = nc.dram_tensor("w2t", [128, 9, 128], BF16,
                           kind="ExternalInput").ap()
    w3_in = nc.dram_tensor("w3t", [128, 9, 128], FP8,
                           kind="ExternalInput").ap()
    id_in = nc.dram_tensor("ident", [128, 128], BF16, kind="ExternalInput").ap()
    gp_in = nc.dram_tensor("gsum", [128, 128], F32, kind="ExternalInput").ap()
    mk_in = nc.dram_tensor("mask4", [128, 4], F32, kind="ExternalInput").ap()
    out_d = nc.dram_tensor("out", [128, H, W], F32, kind="ExternalOutput").ap()
    avp_d = nc.dram_tensor("avp", [128, H, W], BF16).ap()   # scratch, y layout

    with tile.TileContext(nc) as tc:
        consts = tc.alloc_tile_pool(name="consts", bufs=1)
        w1t = consts.tile([128, 9, 128], BF16, tag="w1t")
        w2t = consts.tile([128, 9, 128], BF16, tag="w2t")
        w3t = consts.tile([128, 9, 128], FP8, tag="w3t")
        ident = consts.tile([128, 128], BF16, tag="ident")
        gsum = consts.tile([128, 128], F32, tag="gsum")
        mask4 = consts.tile([128, 4], F32, tag="mask4")
        nc.sync.dma_start(out=w1t[:], in_=w1_in[:])
        nc.sync.dma_start(out=w2t[:], in_=w2_in[:])
        nc.sync.dma_start(out=w3t[:], in_=w3_in[:])
        nc.sync.dma_start(out=ident[:], in_=id_in[:])
        nc.sync.dma_start(out=gsum[:], in_=gp_in[:])
        nc.sync.dma_start(out=mask4[:], in_=mk_in[:])

        small = tc.alloc_tile_pool(name="small", bufs=1)
        stats_cols = small.tile([128, NS, 4, 2], F32, tag="stats_cols")
        sloc = small.tile([128, 8], F32, tag="sloc")
        sglob = small.tile([128, 8], F32, tag="sglob")
        scalars = small.tile([128, 16], F32, tag="scalars")
        msb = small.tile([128, 8], F32, tag="msb")
        sel = small.tile([128, 4], F32, tag="sel")

        kern_pool = tc.alloc_tile_pool(name="kern", bufs=1)
        kern8 = kern_pool.tile([128, 9, 128], FP8, tag="kern8")

        a3_pool = tc.alloc_tile_pool(name="a3p", bufs=1)
        a3p = a3_pool.tile([128, H + 2, XW], FP8, tag="a3p")
        # zero the pad border of a3p once
        nc.vector.memset(_rap(a3p[:], [[1, XW]]), 0.0)                    # row 0
        nc.vector.memset(_rap(a3p[:], [[1, XW]], (H + 1) * XW), 0.0)      # row H+1
        nc.vector.memset(_rap(a3p[:], [[XW, H + 2]]), 0.0)                # col 0
        nc.vector.memset(_rap(a3p[:], [[XW, H + 2]], W + 1), 0.0)         # col W+1

        attn_psp = tc.alloc_tile_pool(name="attn_ps", bufs=1, space="PSUM")
        attn_ps = attn_psp.tile([128, 9 * 128], F32, tag="attn")

        # ---------------- pass A: static convs + attention accumulation ------
        pa_x = tc.alloc_tile_pool(name="pa_x", bufs=2)
        pa_g = tc.alloc_tile_pool(name="pa_g", bufs=2)
        pa_t = tc.alloc_tile_pool(name="pa_t", bufs=4)
        pa_cps = tc.alloc_tile_pool(name="pa_cps", bufs=3, space="PSUM")
        pa_tps = tc.alloc_tile_pool(name="pa_tps", bufs=2, space="PSUM")

        a1gs = {}
        a2gs = {}

        def conv_strip(s):
            y0 = s * R
            xsb = pa_x.tile([128, R + 2, XW], BF16, tag="xsb")
            nc.sync.dma_start(out=xsb[:], in_=xb_in[:, y0:y0 + R + 2, :])
            xs8 = pa_x.tile([128, R + 2, XW], FP8, tag="xs8")
            nc.sync.dma_start(out=xs8[:], in_=x8_in[:, y0:y0 + R + 2, :])
            a1g = pa_g.tile([128, 9, P], BF16, tag="a1g")
            a2g = pa_g.tile([128, 9, P], BF16, tag="a2g")
            a1gs[s] = a1g
            a2gs[s] = a2g
            for wt, gdst, xt, fp8 in ((w1t, a1g, xsb, False),
                                      (w2t, a2g, xsb, False),
                                      (w3t, None, xs8, DR_CONV)):
                for t in range(NT):
                    cps = pa_cps.tile([128, 2 * W], F32, tag="cps")
                    _conv_tile(nc, cps, wt, xt, 2 * t * XW, W, XW, fp8)
                    if gdst is not None:
                        # scatter rows (2t, 2t+1) into subgrid-major layout
                        ya, yb = 2 * t, 2 * t + 1
                        ha, ra = ya % 3, ya // 3
                        hb, rb = yb % 3, yb // 3
                        offa = (3 * ha) * P + ra * Wq
                        sd = (3 * hb) * P + rb * Wq - offa
                        nc.scalar.copy(
                            out=_rap(gdst[:], [[sd, 2], [P, 3], [1, Wq]],
                                     offa),
                            in_=_rap(cps[:], [[W, 2], [1, 3], [3, Wq]]))
                    else:
                        nc.scalar.copy(
                            out=a3p[:, 1 + y0 + 2 * t:1 + y0 + 2 * t + 2,
                                    1:1 + W],
                            in_=_rap(cps[:], [[W, 2], [1, W]]))

        def attn_strip(s):
            a1g, a2g = a1gs.pop(s), a2gs.pop(s)
            for hw in range(9):
                for c0 in range(0, P, 256):
                    t4 = pa_tps.tile([128, 4, 128], BF16, tag="tps")
                    for j, (g, cc) in enumerate(((a1g, c0), (a2g, c0),
                                                 (a1g, c0 + 128),
                                                 (a2g, c0 + 128))):
                        nc.tensor.transpose(t4[:, j, :], g[:, hw, cc:cc + 128],
                                            ident[:])
                    aT4 = pa_t.tile([128, 4, 128], BF16, tag="aT")
                    nc.vector.tensor_copy(aT4[:], t4[:])
                    for j in range(2):
                        nc.tensor.matmul(
                            attn_ps[:, hw * 128:(hw + 1) * 128],
                            aT4[:, 2 * j + 1, :], aT4[:, 2 * j, :],
                            start=(s == 0 and c0 == 0 and j == 0
                                   and hw in (0, 4, 8)),
                            stop=(s == NS - 1 and c0 + 256 >= P and j == 1
                                  and hw in (3, 7, 8)),
                            skip_group_check=True)

        # skewed pipeline: the scalar/vector scatter of strip s drains while
        # PE runs the convs of strip s+1.
        for s in range(NS):
            conv_strip(s)
            if s > 0:
                attn_strip(s - 1)
        attn_strip(NS - 1)

        pa_tps.release(); pa_cps.release()
        pa_t.release(); pa_g.release(); pa_x.release()

        # ---------------- softmax + kern transposes -------------------------
        if level >= 2:
            sm_pool = tc.alloc_tile_pool(name="smx", bufs=1)
            attn_sb = sm_pool.tile([128, 9 * 128], F32, tag="attn_sb")
            nc.vector.tensor_copy(attn_sb[:], attn_ps[:])
            attn_psp.release()
            k_tps = tc.alloc_tile_pool(name="k_tps", bufs=2, space="PSUM")
            mx = scalars[:, 0:1]
            nmx = scalars[:, 1:2]
            ssum = scalars[:, 2:3]
            rsum = scalars[:, 3:4]
            nc.vector.reduce_max(mx, attn_sb[:], axis=mybir.AxisListType.X)
            nc.vector.tensor_scalar_mul(nmx, mx, -SCL)
            esb = sm_pool.tile([128, 9 * 128], F32, tag="esb")
            nc.scalar.activation(esb[:], attn_sb[:],
                                 mybir.ActivationFunctionType.Exp,
                                 bias=nmx, scale=SCL)
            nc.vector.reduce_sum(ssum, esb[:], axis=mybir.AxisListType.X)
            nc.vector.reciprocal(rsum, ssum)
            sm_bf = sm_pool.tile([128, 9 * 128], BF16, tag="sm_bf")
            nc.vector.tensor_scalar_mul(sm_bf[:], esb[:], rsum)
            for hw in range(9):
                tp = k_tps.tile([128, 128], BF16, tag="ktp")
                nc.tensor.transpose(tp[:], sm_bf[:, hw * 128:(hw + 1) * 128],
                                    ident[:])
                nc.vector.tensor_copy(kern8[:, hw, :], tp[:])
            k_tps.release(); sm_pool.release()
        else:
            attn_psp.release()

        # ---------------- pass B: dynamic conv + stats + y scatter ----------
        if level >= 3:
            pb_av = tc.alloc_tile_pool(name="pb_av", bufs=2)
            pb_sq = tc.alloc_tile_pool(name="pb_sq", bufs=2)
            pb_cps = tc.alloc_tile_pool(name="pb_cps", bufs=3, space="PSUM")
            avs = {}

            def dyn_strip(s):
                y0 = s * R
                # av parity-split strip: av_sp[c, 2i+j, p, q] = av[c, 2p+i, 2q+j]
                av_sp = pb_av.tile([128, 4, SR, SQ], BF16, tag="av_sp")
                avs[s] = av_sp
                for t in range(NT):
                    cps = pb_cps.tile([128, 2 * W], F32, tag="cps2")
                    _conv_tile(nc, cps, kern8, a3p, (y0 + 2 * t) * XW, W, XW,
                               DR_CONV)
                    eng = nc.scalar.copy if t % 2 == 0 else nc.vector.tensor_copy
                    eng(
                        _rap(av_sp[:], [[2 * SR * SQ, 2], [SR * SQ, 2],
                                        [1, SQ]], t * SQ),
                        _rap(cps[:], [[W, 2], [1, 2], [2, SQ]]))

            def stats_strip(s):
                av_sp = avs.pop(s)
                sq = pb_sq.tile([128, SR * SQ], BF16, tag="sq")
                for pi in range(4):
                    psrc = _rap(av_sp[:], [[1, SR * SQ]], pi * SR * SQ)
                    nc.vector.reduce_sum(stats_cols[:, s, pi, 0:1], psrc,
                                         axis=mybir.AxisListType.X)
                    nc.vector.scalar_tensor_tensor(
                        out=sq[:], in0=psrc, scalar=1.0, in1=psrc,
                        op0=mybir.AluOpType.mult, op1=mybir.AluOpType.mult,
                        accum_out=stats_cols[:, s, pi, 1:2])
                # feature_map_stack fold: av[32*c2+c1, par, p, q]
                #   -> avp[4*c1+par, S*(c2>>1)+s*SR+p, S*(c2&1)+q]
                for pi in range(4):
                    for c2 in range(4):
                        qsrc = _rap(av_sp[32 * c2:32 * (c2 + 1)],
                                    [[SQ, SR], [1, SQ]],
                                    pi * SR * SQ)
                        dst = bass.AP(
                            tensor=avp_d.tensor,
                            offset=(pi * H * W + (c2 >> 1) * S * W
                                    + (c2 & 1) * SQ + (s * SR) * W),
                            ap=[[4 * H * W, 32], [W, SR], [1, SQ]])
                        nc.sync.dma_start(out=dst, in_=qsrc)

            for s in range(NS):
                dyn_strip(s)
                if s > 0:
                    stats_strip(s - 1)
            stats_strip(NS - 1)
            pb_cps.release()
            pb_sq.release()
            pb_av.release()
        a3_pool.release(); kern_pool.release()

        # ---------------- AllReduce of BN partial sums ----------------------
        if level >= 4:
            nc.vector.reduce_sum(
                _rap(sloc[:], [[2, 4], [1, 2]]),
                _rap(stats_cols[:], [[2, 4], [1, 2], [8, NS]]),
                axis=mybir.AxisListType.X)
            cc_pool = tc.alloc_tile_pool(name="ccd", bufs=1, space="DRAM")
            cc_in = cc_pool.tile([128, 8], F32, tag="cc_in")
            cc_out = cc_pool.tile([128, 8], F32, tag="cc_out")
            nc.gpsimd.dma_start(out=cc_in[:], in_=sloc[:])
            nc.gpsimd.collective_compute(
                "AllReduce", mybir.AluOpType.add,
                replica_groups=[list(range(n_cores))],
                ins=[cc_in.opt()], outs=[cc_out.opt()])
            nc.gpsimd.dma_start(out=sglob[:], in_=cc_out[:])

        # ---------------- pass C: out = cm*x + sc*y + bb ---------------------
        if level >= 5:
            pc_x = tc.alloc_tile_pool(name="pc_x", bufs=2)
            pc_a = tc.alloc_tile_pool(name="pc_a", bufs=2)
            pc_t = tc.alloc_tile_pool(name="pc_t", bufs=2)
            pc_o = tc.alloc_tile_pool(name="pc_o", bufs=2)

            def c_load(s):
                y0 = s * R
                x_s = pc_x.tile([128, R * W], F32, tag="x_s")
                nc.sync.dma_start(out=x_s[:],
                                  in_=x_in[:, 1 + y0:1 + y0 + R, 1:1 + W])
                av_s = pc_a.tile([128, R * W], BF16, tag="av_s")
                nc.sync.dma_start(out=av_s[:], in_=avp_d[:, y0:y0 + R, :])
                return x_s, av_s

            # prefetch strips; independent of the collective, so these DMAs
            # hide its latency
            loads = {}
            for s2 in range(min(2, NS)):
                loads[s2] = c_load(s2)

            if level >= 4:
                # ------------ BN coefficients (per out-channel) -------------
                bn_ps = tc.alloc_tile_pool(name="bn_ps", bufs=1, space="PSUM")
                gps = bn_ps.tile([128, 8], F32, tag="gps")
                nc.tensor.matmul(gps[:], gsum[:], sglob[:], start=True,
                                 stop=True)
                nc.vector.tensor_copy(msb[:], gps[:])
                bn_ps.release()
                mean = scalars[:, 4:5]
                e2 = scalars[:, 5:6]
                msq = scalars[:, 6:7]
                var = scalars[:, 7:8]
                sd = scalars[:, 8:9]
                rstd = scalars[:, 9:10]
                sc = scalars[:, 10:11]
                bb0 = scalars[:, 11:12]
                bb = scalars[:, 12:13]
                nc.vector.tensor_mul(sel[:], _rap(msb[:], [[2, 4]]), mask4[:])
                nc.vector.reduce_sum(mean, sel[:], axis=mybir.AxisListType.X)
                nc.vector.tensor_scalar_mul(mean, mean, 1.0 / N_TOT)
                nc.vector.tensor_mul(sel[:], _rap(msb[:], [[2, 4]], 1),
                                     mask4[:])
                nc.vector.reduce_sum(e2, sel[:], axis=mybir.AxisListType.X)
                nc.vector.tensor_scalar_mul(e2, e2, 1.0 / N_TOT)
                nc.vector.tensor_mul(msq, mean, mean)
                nc.vector.tensor_tensor(out=var, in0=e2, in1=msq,
                                        op=mybir.AluOpType.subtract)
                eps_ap = scalars[:, 13:14]
                nc.vector.memset(eps_ap, EPS)
                nc.scalar.activation(sd, var,
                                     mybir.ActivationFunctionType.Sqrt,
                                     bias=eps_ap)
                nc.vector.reciprocal(rstd, sd)
                nc.vector.tensor_scalar_mul(sc, rstd, NORM_SCALE)
                nc.vector.tensor_mul(bb0, mean, sc)
                nc.vector.tensor_scalar_mul(bb, bb0, -1.0)

            for s in range(NS):
                x_s, av_s = loads.pop(s)
                if s + 2 < NS:
                    loads[s + 2] = c_load(s + 2)
                tv = pc_t.tile([128, R * W], F32, tag="tv")
                if s % 2 == 0:
                    nc.scalar.activation(
                        tv[:], av_s[:], mybir.ActivationFunctionType.Identity,
                        bias=bb, scale=sc)
                else:
                    nc.vector.tensor_scalar(
                        out=tv[:], in0=av_s[:], scalar1=sc, scalar2=bb,
                        op0=mybir.AluOpType.mult, op1=mybir.AluOpType.add)
                o_s = pc_o.tile([128, R * W], F32, tag="o_s")
                nc.vector.scalar_tensor_tensor(
                    out=o_s[:], in0=x_s[:], scalar=float(cm), in1=tv[:],
                    op0=mybir.AluOpType.mult, op1=mybir.AluOpType.add)
                nc.sync.dma_start(out=out_d[:, s * R:(s + 1) * R, :],
                                  in_=o_s[:])
            pc_o.release(); pc_t.release(); pc_a.release(); pc_x.release()

        if level >= 4:
            cc_pool.release()
        small.release()
        consts.release()

    nc.compile()
    return nc


def _prep_wt(w, dt, permute_out=False):
    """[Co,Ci,3,3] -> lhsT layout [Ci, 9, Co] (optionally out-chan permuted)."""
    wt = np.ascontiguousarray(w.transpose(1, 2, 3, 0).reshape(128, 9, 128))
    if permute_out:
        p = np.arange(128)
        co_of_p = 4 * (p % 32) + p // 32     # partition p holds channel co_of_p
        wt = np.ascontiguousarray(wt[:, :, co_of_p])
    return np.ascontiguousarray(wt.astype(dt))


def make_const_inputs(w1, w2, w3):
    import ml_dtypes
    E4 = ml_dtypes.float8_e4m3
    BF = ml_dtypes.bfloat16
    ident_bf = np.eye(128, dtype=np.float32).astype(BF)
    p = np.arange(128)
    # gsum[p_src, C']: sum av partitions with p_src%32 == C'//4
    gsum = (p[:, None] % 32 == p[None, :] // 4).astype(np.float32)
    mask4 = (p[:, None] % 4 == np.arange(4)[None, :]).astype(np.float32)
    return {
        "w1t": _prep_wt(np.asarray(w1, np.float32), BF),
        "w2t": _prep_wt(np.asarray(w2, np.float32), BF, permute_out=True),
        "w3t": _prep_wt(np.asarray(w3, np.float32), E4),
        "ident": ident_bf,
        "gsum": gsum,
        "mask4": mask4,
    }


def pad_x(x_sample):
    return np.pad(x_sample, ((0, 0), (1, 1), (1, 1)))


def make_in_maps(x, w1, w2, w3):
    import ml_dtypes
    consts = make_const_inputs(w1, w2, w3)
    in_maps = []
    for b in range(x.shape[0]):
        xp = pad_x(np.asarray(x[b], np.float32))
        m = dict(consts, x=xp,
                 xb=np.ascontiguousarray(xp.astype(ml_dtypes.bfloat16)),
                 x8=np.ascontiguousarray(xp.astype(ml_dtypes.float8_e4m3)))
        in_maps.append(m)
    return in_maps


_CACHE = {}


def kernel(x, w1, w2, w3, conv_momentum):
    from concourse.bass_utils import run_bass_kernel_spmd

    x = np.asarray(x, np.float32)
    B, Ci, H, W = x.shape
    cm = float(np.asarray(conv_momentum))
    key = (H, W, B, cm)
    if key not in _CACHE:
        _CACHE[key] = build_nc(H, W, 24, B, cm)
    nc = _CACHE[key]
    in_maps = make_in_maps(x, w1, w2, w3)
    res = run_bass_kernel_spmd(nc, in_maps, list(range(B)))
    out = np.stack(
        [np.asarray(res.results[b]["out"]).reshape(128, H, W) for b in range(B)],
        axis=0)
    return out.astype(np.float32)


# revision 13
# speedup vs baseline: 1.5016x; 1.1651x over previous
"""Trainium2 Bass kernel for nn_AttnConv2d (attention-conv + dynamic conv + BN).

Math (per sample b):
  a1 = conv3x3(x, w1); a2 = conv3x3(x, w2); a3 = conv3x3(x, w3)     (SAME pad)
  attn[h,w,i,o] = sum_{p,q} a1[i,3p+h,3q+w] * a2[o,3p+h,3q+w]
  kern[o,:,:,:] = softmax(attn[.,.,.,o] / sqrt(Ci*9))
  av = conv3x3(a3, kern[b])                                         (per-sample kernel)
  y  = feature_map_stack(av)   (pure spatial/channel permutation)
  out = cm * x + NORM_SCALE * (y - mean_y) * rsqrt(var_y + eps)     (batch stats)

Sharding: data-parallel over batch, 1 sample per core, 8 cores.  The only
cross-core exchange is an AllReduce of the per-channel BN partial sums.

Implementation notes:
  - x arrives host-padded ([128, H+2, W+2]) in bf16 (a1/a2 convs) and fp8
    (a3 conv), so convs are 9 shifted accumulating matmuls into PSUM with
    no on-device edge handling.  a3 and the dynamic conv run fp8 with
    DoubleRow perf mode (two kernel offsets per matmul, K=256); fp8 there
    costs ~1e-2 rel err (attention path must stay bf16: fp8 a1/a2 alone
    is 1.8e-2).
  - attention contraction needs positions on the partition axis: conv
    outputs are scatter-copied to subgrid-major SBUF (bf16), PE-transposed
    in 128-position chunks, then accumulated into a persistent PSUM tile.
  - a2's output-channel order is permuted host-side (partition p holds
    channel 4*(p%32) + p//32) so feature_map_stack becomes a
    per-(partition,parity) affine map on av.
  - feature_map_stack is applied strip-by-strip during pass B as
    SBUF->SBUF scatter DMAs into a y-layout tile, so the permutation's
    descriptor cost overlaps the dynamic-conv compute; pass C then runs
    entirely on clean contiguous DMAs (x strip load, out strip store).
  - BN group-of-4 partition sums are a tiny 0/1 matmul; x prefetch is
    issued before the AllReduce so the collective latency is hidden.
  - strip pipeline is skewed: transposes+attention for strip s issue after
    the convs of strip s+1, so the scalar-engine scatter never stalls PE.
"""

import os
import sys

for _p in ("/opt/trn_rl_repo", "/root/.axon_site/_ro/trn_rl_repo"):
    if os.path.isdir(_p) and _p not in sys.path:
        sys.path.insert(0, _p)
        break

import numpy as np

import concourse.bass as bass
import concourse.bacc as bacc
import concourse.tile as tile
from concourse import mybir

F32 = mybir.dt.float32
BF16 = mybir.dt.bfloat16
FP8 = mybir.dt.float8e4
DR = mybir.MatmulPerfMode.DoubleRow

EPS = 1e-5
NORM_SCALE = 0.1816
CI = 128

# a3/dynamic-conv path runs fp8 (attention path stays bf16); DR_CONV
# selects DoubleRow pairing (2 offsets per matmul) vs plain fp8 matmuls.
DR_CONV = os.environ.get("DR_CONV", "1") == "1"


def _rap(base, dims, off=0):
    """Raw AP on the same tensor as `base` (keeps base's partition dim)."""
    return bass.AP(tensor=base.tensor, offset=base.offset + off,
                   ap=[base.ap[0]] + [list(d) for d in dims])


def _conv_tile(nc, cps, wt, xs, base, W, XW, dr):
    """Accumulate the 9-offset conv into PSUM cps [128, 2*W] for the row
    pair whose top-left (unshifted) element is at linear offset `base`
    within xs (a padded [128, rows, XW] tile)."""
    if not dr:
        for k in range(9):
            dy, dx = divmod(k, 3)
            rhs = _rap(xs[:], [[XW, 2], [1, W]], base + dy * XW + dx)
            nc.tensor.matmul(cps[:, :], wt[:, k, :], rhs,
                             start=(k == 0), stop=(k == 8))
        return
    deltas = [dy * XW + dx for dy in range(3) for dx in range(3)]
    for j in range(4):
        da, db = deltas[2 * j], deltas[2 * j + 1]
        rhs = _rap(xs[:], [[db - da, 2], [XW, 2], [1, W]], base + da)
        nc.tensor.matmul(cps[:, :], wt[:, 2 * j:2 * j + 2, :], rhs,
                         start=(j == 0), stop=False, perf_mode=DR,
                         skip_group_check=True)
    rhs = _rap(xs[:], [[XW, 2], [1, W]], base + deltas[8])
    nc.tensor.matmul(cps[:, :], wt[:, 8, :], rhs,
                     start=False, stop=True, skip_group_check=True)


def build_nc(H, W, R, n_cores, cm, level=5):
    """Build the per-core Bass kernel. R = strip rows (div by 6, even)."""
    assert H % R == 0 and R % 6 == 0 and W % 6 == 0
    NS = H // R                      # strips
    Wq = W // 3                      # attn subgrid cols
    P = (R // 3) * Wq                # attn positions per offset per strip
    S = H // 2                       # quadrant size of feature_map_stack
    NT = R // 2                      # psum tiles (2 rows) per strip
    SR = R // 2                      # subgrid rows per strip (parity space)
    SQ = W // 2                      # subgrid cols (parity space)
    N_TOT = float(n_cores * H * W)   # BN count per channel
    SCL = 1.0 / float(np.sqrt(CI * 9))
    XW = W + 2                       # padded row pitch

    nc = bacc.Bacc("TRN2", target_bir_lowering=False, debug=False,
                   num_devices=n_cores)

    xb_in = nc.dram_tensor("xb", [128, H + 2, XW], BF16,
                           kind="ExternalInput").ap()
    x8_in = nc.dram_tensor("x8", [128, H + 2, XW], FP8,
                           kind="ExternalInput").ap()
    w1_in = nc.dram_tensor("w1t", [128, 9, 128], BF16,
                           kind="ExternalInput").ap()
    w2_in = nc.dram_tensor("w2t", [128, 9, 128], BF16,
                           kind="ExternalInput").ap()
    w3_in = nc.dram_tensor("w3t", [128, 9, 128], FP8,
                           kind="ExternalInput").ap()
    id_in = nc.dram_tensor("ident", [128, 128], BF16, kind="ExternalInput").ap()
    gp_in = nc.dram_tensor("gsum", [128, 128], F32, kind="ExternalInput").ap()
    mk_in = nc.dram_tensor("mask4", [128, 4], F32, kind="ExternalInput").ap()
    out_d = nc.dram_tensor("out", [128, H, W], F32, kind="ExternalOutput").ap()
    avp_d = nc.dram_tensor("avp", [128, H, W], BF16).ap()   # scratch, y layout

    with tile.TileContext(nc) as tc:
        consts = tc.alloc_tile_pool(name="consts", bufs=1)
        w1t = consts.tile([128, 9, 128], BF16, tag="w1t")
        w2t = consts.tile([128, 9, 128], BF16, tag="w2t")
        w3t = consts.tile([128, 9, 128], FP8, tag="w3t")
        ident = consts.tile([128, 128], BF16, tag="ident")
        gsum = consts.tile([128, 128], F32, tag="gsum")
        mask4 = consts.tile([128, 4], F32, tag="mask4")
        nc.sync.dma_start(out=w1t[:], in_=w1_in[:])
        nc.sync.dma_start(out=w2t[:], in_=w2_in[:])
        nc.sync.dma_start(out=w3t[:], in_=w3_in[:])
        nc.sync.dma_start(out=ident[:], in_=id_in[:])
        nc.sync.dma_start(out=gsum[:], in_=gp_in[:])
        nc.sync.dma_start(out=mask4[:], in_=mk_in[:])

        small = tc.alloc_tile_pool(name="small", bufs=1)
        stats_cols = small.tile([128, NS, 4, 2], F32, tag="stats_cols")
        sloc = small.tile([128, 8], F32, tag="sloc")
        sglob = small.tile([128, 8], F32, tag="sglob")
        scalars = small.tile([128, 16], F32, tag="scalars")
        msb = small.tile([128, 8], F32, tag="msb")
        sel = small.tile([128, 4], F32, tag="sel")

        # xb strips persist from pass A through pass C (residual input)
        pa_xb = tc.alloc_tile_pool(name="pa_xb", bufs=NS)

        kern_pool = tc.alloc_tile_pool(name="kern", bufs=1)
        kern8 = kern_pool.tile([128, 9, 128], FP8, tag="kern8")

        a3_pool = tc.alloc_tile_pool(name="a3p", bufs=1)
        a3p = a3_pool.tile([128, H + 2, XW], FP8, tag="a3p")
        # zero the pad border of a3p once
        nc.vector.memset(_rap(a3p[:], [[1, XW]]), 0.0)                    # row 0
        nc.vector.memset(_rap(a3p[:], [[1, XW]], (H + 1) * XW), 0.0)      # row H+1
        nc.vector.memset(_rap(a3p[:], [[XW, H + 2]]), 0.0)                # col 0
        nc.vector.memset(_rap(a3p[:], [[XW, H + 2]], W + 1), 0.0)         # col W+1

        attn_psp = tc.alloc_tile_pool(name="attn_ps", bufs=1, space="PSUM")
        attn_ps = attn_psp.tile([128, 9 * 128], F32, tag="attn")

        # ---------------- pass A: static convs + attention accumulation ------
        pa_x = tc.alloc_tile_pool(name="pa_x", bufs=2)
        pa_g = tc.alloc_tile_pool(name="pa_g", bufs=2)
        pa_t = tc.alloc_tile_pool(name="pa_t", bufs=4)
        pa_cps = tc.alloc_tile_pool(name="pa_cps", bufs=3, space="PSUM")
        pa_tps = tc.alloc_tile_pool(name="pa_tps", bufs=2, space="PSUM")

        a1gs = {}
        a2gs = {}
        xsbs = {}

        def conv_strip(s):
            y0 = s * R
            xsb = pa_xb.tile([128, R + 2, XW], BF16, tag="xsb")
            xsbs[s] = xsb
            nc.sync.dma_start(out=xsb[:], in_=xb_in[:, y0:y0 + R + 2, :])
            xs8 = pa_x.tile([128, R + 2, XW], FP8, tag="xs8")
            nc.sync.dma_start(out=xs8[:], in_=x8_in[:, y0:y0 + R + 2, :])
            a1g = pa_g.tile([128, 9, P], BF16, tag="a1g")
            a2g = pa_g.tile([128, 9, P], BF16, tag="a2g")
            a1gs[s] = a1g
            a2gs[s] = a2g
            for wt, gdst, xt, fp8 in ((w1t, a1g, xsb, False),
                                      (w2t, a2g, xsb, False),
                                      (w3t, None, xs8, DR_CONV)):
                for t in range(NT):
                    cps = pa_cps.tile([128, 2 * W], F32, tag="cps")
                    _conv_tile(nc, cps, wt, xt, 2 * t * XW, W, XW, fp8)
                    if gdst is not None:
                        # scatter rows (2t, 2t+1) into subgrid-major layout
                        ya, yb = 2 * t, 2 * t + 1
                        ha, ra = ya % 3, ya // 3
                        hb, rb = yb % 3, yb // 3
                        offa = (3 * ha) * P + ra * Wq
                        sd = (3 * hb) * P + rb * Wq - offa
                        nc.scalar.copy(
                            out=_rap(gdst[:], [[sd, 2], [P, 3], [1, Wq]],
                                     offa),
                            in_=_rap(cps[:], [[W, 2], [1, 3], [3, Wq]]))
                    else:
                        nc.scalar.copy(
                            out=a3p[:, 1 + y0 + 2 * t:1 + y0 + 2 * t + 2,
                                    1:1 + W],
                            in_=_rap(cps[:], [[W, 2], [1, W]]))

        def attn_strip(s):
            a1g, a2g = a1gs.pop(s), a2gs.pop(s)
            for hw in range(9):
                for c0 in range(0, P, 256):
                    t4 = pa_tps.tile([128, 4, 128], BF16, tag="tps")
                    for j, (g, cc) in enumerate(((a1g, c0), (a2g, c0),
                                                 (a1g, c0 + 128),
                                                 (a2g, c0 + 128))):
                        nc.tensor.transpose(t4[:, j, :], g[:, hw, cc:cc + 128],
                                            ident[:])
                    aT4 = pa_t.tile([128, 4, 128], BF16, tag="aT")
                    nc.vector.tensor_copy(aT4[:], t4[:])
                    for j in range(2):
                        nc.tensor.matmul(
                            attn_ps[:, hw * 128:(hw + 1) * 128],
                            aT4[:, 2 * j + 1, :], aT4[:, 2 * j, :],
                            start=(s == 0 and c0 == 0 and j == 0
                                   and hw in (0, 4, 8)),
                            stop=(s == NS - 1 and c0 + 256 >= P and j == 1
                                  and hw in (3, 7, 8)),
                            skip_group_check=True)

        # skewed pipeline: the scalar/vector scatter of strip s drains while
        # PE runs the convs of strip s+1.
        for s in range(NS):
            conv_strip(s)
            if s > 0:
                attn_strip(s - 1)
        attn_strip(NS - 1)

        pa_tps.release(); pa_cps.release()
        pa_t.release(); pa_g.release(); pa_x.release()
        # pa_xb stays live: pass C reuses the xb strips for the residual

        # ---------------- softmax + kern transposes -------------------------
        if level >= 2:
            sm_pool = tc.alloc_tile_pool(name="smx", bufs=1)
            attn_sb = sm_pool.tile([128, 9 * 128], F32, tag="attn_sb")
            nc.vector.tensor_copy(attn_sb[:], attn_ps[:])
            attn_psp.release()
            k_tps = tc.alloc_tile_pool(name="k_tps", bufs=2, space="PSUM")
            mx = scalars[:, 0:1]
            nmx = scalars[:, 1:2]
            ssum = scalars[:, 2:3]
            rsum = scalars[:, 3:4]
            nc.vector.reduce_max(mx, attn_sb[:], axis=mybir.AxisListType.X)
            nc.vector.tensor_scalar_mul(nmx, mx, -SCL)
            esb = sm_pool.tile([128, 9 * 128], F32, tag="esb")
            nc.scalar.activation(esb[:], attn_sb[:],
                                 mybir.ActivationFunctionType.Exp,
                                 bias=nmx, scale=SCL)
            nc.vector.reduce_sum(ssum, esb[:], axis=mybir.AxisListType.X)
            nc.vector.reciprocal(rsum, ssum)
            sm_bf = sm_pool.tile([128, 9 * 128], BF16, tag="sm_bf")
            nc.vector.tensor_scalar_mul(sm_bf[:], esb[:], rsum)
            for hw in range(9):
                tp = k_tps.tile([128, 128], BF16, tag="ktp")
                nc.tensor.transpose(tp[:], sm_bf[:, hw * 128:(hw + 1) * 128],
                                    ident[:])
                nc.vector.tensor_copy(kern8[:, hw, :], tp[:])
            k_tps.release(); sm_pool.release()
        else:
            attn_psp.release()

        # ---------------- pass B: dynamic conv + stats + y scatter ----------
        if level >= 3:
            pb_av = tc.alloc_tile_pool(name="pb_av", bufs=3)
            pb_sq = tc.alloc_tile_pool(name="pb_sq", bufs=2)
            pb_cps = tc.alloc_tile_pool(name="pb_cps", bufs=3, space="PSUM")
            avs = {}

            def dyn_strip(s):
                y0 = s * R
                # av parity-split strip: av_sp[c, 2i+j, p, q] = av[c, 2p+i, 2q+j]
                av_sp = pb_av.tile([128, 4, SR, SQ], BF16, tag="av_sp")
                avs[s] = av_sp
                for t in range(NT):
                    cps = pb_cps.tile([128, 2 * W], F32, tag="cps2")
                    _conv_tile(nc, cps, kern8, a3p, (y0 + 2 * t) * XW, W, XW,
                               DR_CONV)
                    nc.scalar.copy(
                        out=_rap(av_sp[:], [[2 * SR * SQ, 2], [SR * SQ, 2],
                                            [1, SQ]], t * SQ),
                        in_=_rap(cps[:], [[W, 2], [1, 2], [2, SQ]]))

            def stats_strip(s):
                av_sp = avs.pop(s)
                sq = pb_sq.tile([128, SR * SQ], BF16, tag="sq")
                for pi in range(4):
                    psrc = _rap(av_sp[:], [[1, SR * SQ]], pi * SR * SQ)
                    nc.vector.reduce_sum(stats_cols[:, s, pi, 0:1], psrc,
                                         axis=mybir.AxisListType.X)
                    nc.vector.scalar_tensor_tensor(
                        out=sq[:], in0=psrc, scalar=1.0, in1=psrc,
                        op0=mybir.AluOpType.mult, op1=mybir.AluOpType.mult,
                        accum_out=stats_cols[:, s, pi, 1:2])
                # feature_map_stack fold: av[32*c2+c1, par, p, q]
                #   -> avp[4*c1+par, S*(c2>>1)+s*SR+p, S*(c2&1)+q]
                for pi in range(4):
                    for c2 in range(4):
                        qsrc = _rap(av_sp[32 * c2:32 * (c2 + 1)],
                                    [[SQ, SR], [1, SQ]],
                                    pi * SR * SQ)
                        dst = bass.AP(
                            tensor=avp_d.tensor,
                            offset=(pi * H * W + (c2 >> 1) * S * W
                                    + (c2 & 1) * SQ + (s * SR) * W),
                            ap=[[4 * H * W, 32], [W, SR], [1, SQ]])
                        nc.sync.dma_start(out=dst, in_=qsrc)

            for s in range(NS):
                dyn_strip(s)
                if s > 0:
                    stats_strip(s - 1)
            stats_strip(NS - 1)
            pb_cps.release()
            pb_sq.release()
            pb_av.release()
        a3_pool.release(); kern_pool.release()

        # ---------------- AllReduce of BN partial sums ----------------------
        if level >= 4:
            nc.vector.reduce_sum(
                _rap(sloc[:], [[2, 4], [1, 2]]),
                _rap(stats_cols[:], [[2, 4], [1, 2], [8, NS]]),
                axis=mybir.AxisListType.X)
            cc_pool = tc.alloc_tile_pool(name="ccd", bufs=1, space="DRAM")
            cc_in = cc_pool.tile([128, 8], F32, tag="cc_in")
            cc_out = cc_pool.tile([128, 8], F32, tag="cc_out")
            nc.sync.dma_start(out=cc_in[:], in_=sloc[:])
            nc.gpsimd.collective_compute(
                "AllReduce", mybir.AluOpType.add,
                replica_groups=[list(range(n_cores))],
                ins=[cc_in.opt()], outs=[cc_out.opt()])
            nc.sync.dma_start(out=sglob[:], in_=cc_out[:])

        # ---------------- pass C: out = cm*x + sc*y + bb ---------------------
        if level >= 5:
            pc_a = tc.alloc_tile_pool(name="pc_a", bufs=3)
            pc_t = tc.alloc_tile_pool(name="pc_t", bufs=2)
            pc_o = tc.alloc_tile_pool(name="pc_o", bufs=2)

            def c_load(s):
                av_s = pc_a.tile([128, R * W], BF16, tag="av_s")
                nc.sync.dma_start(out=av_s[:], in_=avp_d[:, s * R:s * R + R, :])
                return av_s

            # prefetch strips; independent of the collective, so these DMAs
            # hide its latency
            loads = {}
            for s2 in range(min(3, NS)):
                loads[s2] = c_load(s2)

            if level >= 4:
                # ------------ BN coefficients (per out-channel) -------------
                bn_ps = tc.alloc_tile_pool(name="bn_ps", bufs=1, space="PSUM")
                gps = bn_ps.tile([128, 8], F32, tag="gps")
                nc.tensor.matmul(gps[:], gsum[:], sglob[:], start=True,
                                 stop=True)
                nc.vector.tensor_copy(msb[:], gps[:])
                bn_ps.release()
                mean = scalars[:, 4:5]
                e2 = scalars[:, 5:6]
                msq = scalars[:, 6:7]
                var = scalars[:, 7:8]
                sd = scalars[:, 8:9]
                rstd = scalars[:, 9:10]
                sc = scalars[:, 10:11]
                bb0 = scalars[:, 11:12]
                bb = scalars[:, 12:13]
                nc.vector.tensor_mul(sel[:], _rap(msb[:], [[2, 4]]), mask4[:])
                nc.vector.reduce_sum(mean, sel[:], axis=mybir.AxisListType.X)
                nc.vector.tensor_scalar_mul(mean, mean, 1.0 / N_TOT)
                nc.vector.tensor_mul(sel[:], _rap(msb[:], [[2, 4]], 1),
                                     mask4[:])
                nc.vector.reduce_sum(e2, sel[:], axis=mybir.AxisListType.X)
                nc.vector.tensor_scalar_mul(e2, e2, 1.0 / N_TOT)
                nc.vector.tensor_mul(msq, mean, mean)
                nc.vector.tensor_tensor(out=var, in0=e2, in1=msq,
                                        op=mybir.AluOpType.subtract)
                eps_ap = scalars[:, 13:14]
                nc.vector.memset(eps_ap, EPS)
                nc.scalar.activation(sd, var,
                                     mybir.ActivationFunctionType.Sqrt,
                                     bias=eps_ap)
                nc.vector.reciprocal(rstd, sd)
                nc.vector.tensor_scalar_mul(sc, rstd, NORM_SCALE)
                nc.vector.tensor_mul(bb0, mean, sc)
                nc.vector.tensor_scalar_mul(bb, bb0, -1.0)

            for s in range(NS):
                av_s = loads.pop(s)
                if s + 3 < NS:
                    loads[s + 3] = c_load(s + 3)
                xsb = xsbs.pop(s)
                tv = pc_t.tile([128, R * W], F32, tag="tv")
                nc.scalar.activation(
                    tv[:], av_s[:], mybir.ActivationFunctionType.Identity,
                    bias=bb, scale=sc)
                o_s = pc_o.tile([128, R * W], F32, tag="o_s")
                nc.vector.scalar_tensor_tensor(
                    out=o_s[:], in0=_rap(xsb[:], [[XW, R], [1, W]], XW + 1),
                    scalar=float(cm), in1=tv[:],
                    op0=mybir.AluOpType.mult, op1=mybir.AluOpType.add)
                nc.sync.dma_start(out=out_d[:, s * R:(s + 1) * R, :],
                                  in_=o_s[:])
            pc_o.release(); pc_t.release(); pc_a.release()

        if level >= 4:
            cc_pool.release()
        pa_xb.release()
        small.release()
        consts.release()

    nc.compile()
    return nc


def _prep_wt(w, dt, permute_out=False):
    """[Co,Ci,3,3] -> lhsT layout [Ci, 9, Co] (optionally out-chan permuted)."""
    wt = np.ascontiguousarray(w.transpose(1, 2, 3, 0).reshape(128, 9, 128))
    if permute_out:
        p = np.arange(128)
        co_of_p = 4 * (p % 32) + p // 32     # partition p holds channel co_of_p
        wt = np.ascontiguousarray(wt[:, :, co_of_p])
    return np.ascontiguousarray(wt.astype(dt))


def make_const_inputs(w1, w2, w3):
    import ml_dtypes
    E4 = ml_dtypes.float8_e4m3
    BF = ml_dtypes.bfloat16
    ident_bf = np.eye(128, dtype=np.float32).astype(BF)
    p = np.arange(128)
    # gsum[p_src, C']: sum av partitions with p_src%32 == C'//4
    gsum = (p[:, None] % 32 == p[None, :] // 4).astype(np.float32)
    mask4 = (p[:, None] % 4 == np.arange(4)[None, :]).astype(np.float32)
    return {
        "w1t": _prep_wt(np.asarray(w1, np.float32), BF),
        "w2t": _prep_wt(np.asarray(w2, np.float32), BF, permute_out=True),
        "w3t": _prep_wt(np.asarray(w3, np.float32), E4),
        "ident": ident_bf,
        "gsum": gsum,
        "mask4": mask4,
    }


def pad_x(x_sample):
    return np.pad(x_sample, ((0, 0), (1, 1), (1, 1)))


def make_in_maps(x, w1, w2, w3):
    import ml_dtypes
    consts = make_const_inputs(w1, w2, w3)
    in_maps = []
    for b in range(x.shape[0]):
        xp = pad_x(np.asarray(x[b], np.float32))
        m = dict(consts,
                 xb=np.ascontiguousarray(xp.astype(ml_dtypes.bfloat16)),
                 x8=np.ascontiguousarray(xp.astype(ml_dtypes.float8_e4m3)))
        in_maps.append(m)
    return in_maps


_CACHE = {}


def kernel(x, w1, w2, w3, conv_momentum):
    from concourse.bass_utils import run_bass_kernel_spmd

    x = np.asarray(x, np.float32)
    B, Ci, H, W = x.shape
    cm = float(np.asarray(conv_momentum))
    key = (H, W, B, cm)
    if key not in _CACHE:
        _CACHE[key] = build_nc(H, W, 24, B, cm)
    nc = _CACHE[key]
    in_maps = make_in_maps(x, w1, w2, w3)
    res = run_bass_kernel_spmd(nc, in_maps, list(range(B)))
    out = np.stack(
        [np.asarray(res.results[b]["out"]).reshape(128, H, W) for b in range(B)],
        axis=0)
    return out.astype(np.float32)
